# revision 5
# baseline (speedup 1.0000x reference)
"""Devign GGNN + conv-head kernel for 8 Trainium2 NeuronCores — v2.

Changes vs baseline:
- Adjacency stored fp8-e4m3 (counts <= 3, exact) and SBUF-resident across
  all 8 GGNN steps (was: bf16 re-streamed every step, 134 MB DMA).
- Aggregation matmuls use fp8 DoubleRow (K=256 per instruction): stationary
  Ht tiles are cast to fp8, moving adjacency is fp8.  16 -> 8 matmuls per
  (graph, out-chunk).
- GRU h-side gate matmuls (gh = h @ W_hh^T) use fp8 DoubleRow with a paired
  [128, 2, *] layout for h and W_hh^T.  Weight-side fp8 for wcat/wih is NOT
  used (systematic quantization error too large; measured in numpy sim).
- h master state in bf16 (was f32), GRU elementwise update moved to GpSimd,
  PSUM->SBUF casts split between Vector and Scalar engines.
"""

import numpy as np
import ml_dtypes

import concourse.bass as bass
import concourse.bacc as bacc
import concourse.tile as tile
from concourse import mybir
from concourse._compat import get_trn_type
from concourse.bass_utils import run_bass_kernel_spmd

BF16 = mybir.dt.bfloat16
F32 = mybir.dt.float32
F8 = mybir.dt.float8e4
DR = mybir.MatmulPerfMode.DoubleRow

N_CORES = 8
B = 64                    # total graphs
G = B // N_CORES          # graphs per core = 8
N = 512                   # nodes per graph
NLOC = G * N              # nodes per core = 4096
IN_DIM = 100
OUT = 200                 # hidden dim
NT = 4                    # edge types
N_STEPS = 8
N_CLASSES = 7
CONCAT = IN_DIM + OUT     # 300

OC = [(0, 128), (128, 72)]          # plain 200 rows
AC = [(0, 128), (128, 73)]          # 201 rows incl ones row
CC = [(0, 128), (128, 128), (256, 44)]   # 300 rows (concat branch)
MZC = [(0, 128), (128, 128), (256, 45)]  # 301 rows incl ones
WPAD = 608                # padded col count for whh8 (608 % 16 == 0)


def _build_program():
    nc = bacc.Bacc(get_trn_type() or "TRN2", target_bir_lowering=False)

    # ---------------- DRAM I/O ----------------
    d_feat = nc.dram_tensor("features", (NLOC, IN_DIM), F32, kind="ExternalInput")
    d_adj = nc.dram_tensor("adj", (G, 128, 16, N), F8, kind="ExternalInput")
    d_wcat = nc.dram_tensor("wcat", (OUT + 1, NT * OUT), BF16, kind="ExternalInput")
    d_wih = nc.dram_tensor("wih", (OUT + 1, 3 * OUT), BF16, kind="ExternalInput")
    d_whh8 = nc.dram_tensor("whh8", (128, 2, WPAD), F8, kind="ExternalInput")
    d_c1w = nc.dram_tensor("c1w", (OUT, 3 * OUT), BF16, kind="ExternalInput")
    d_c2w = nc.dram_tensor("c2w", (OUT, OUT), BF16, kind="ExternalInput")
    d_cc1w = nc.dram_tensor("cc1w", (CONCAT, 3 * CONCAT), BF16, kind="ExternalInput")
    d_cc2w = nc.dram_tensor("cc2w", (CONCAT, CONCAT), BF16, kind="ExternalInput")
    d_mlpy = nc.dram_tensor("mlpy", (OUT + 1, N_CLASSES), BF16, kind="ExternalInput")
    d_mlpz = nc.dram_tensor("mlpz", (CONCAT + 1, N_CLASSES), BF16, kind="ExternalInput")
    d_c1b = nc.dram_tensor("c1b", (OUT, 1), F32, kind="ExternalInput")
    d_c2b = nc.dram_tensor("c2b", (OUT, 1), F32, kind="ExternalInput")
    d_cc1b = nc.dram_tensor("cc1b", (CONCAT, 1), F32, kind="ExternalInput")
    d_cc2b = nc.dram_tensor("cc2b", (CONCAT, 1), F32, kind="ExternalInput")
    d_ident = nc.dram_tensor("ident", (128, 128), F32, kind="ExternalInput")
    d_ones8 = nc.dram_tensor("ones8", (1, NLOC), F8, kind="ExternalInput")
    d_out = nc.dram_tensor("out", (G, N_CLASSES), F32, kind="ExternalOutput")

    with tile.TileContext(nc) as tc:
        with (
            tc.tile_pool(name="const", bufs=1) as cpool,
            tc.tile_pool(name="state", bufs=1) as spool,
            tc.tile_pool(name="psall", bufs=8, space="PSUM") as pspool,
        ):
            def ps(p_, n_):
                return pspool.tile([p_, n_], F32, name="ps", tag="ps")

            # ---------------- persistent SBUF tensors ----------------
            wcat = [cpool.tile([sz, NT * OUT], BF16, name=f"wcat{i}")
                    for i, (off, sz) in enumerate(AC)]
            wih = [cpool.tile([sz, 3 * OUT], BF16, name=f"wih{i}")
                   for i, (off, sz) in enumerate(AC)]
            whh8 = cpool.tile([128, 2, WPAD], F8, name="whh8")
            c1w = [cpool.tile([sz, 3 * OUT], BF16, name=f"c1w{i}")
                   for i, (off, sz) in enumerate(OC)]
            c2w = [cpool.tile([sz, OUT], BF16, name=f"c2w{i}")
                   for i, (off, sz) in enumerate(OC)]
            cc1w = [cpool.tile([sz, 3 * CONCAT], BF16, name=f"cc1w{i}")
                    for i, (off, sz) in enumerate(CC)]
            cc2w = [cpool.tile([sz, CONCAT], BF16, name=f"cc2w{i}")
                    for i, (off, sz) in enumerate(CC)]
            mlpy = [cpool.tile([sz, N_CLASSES], BF16, name=f"mlpy{i}")
                    for i, (off, sz) in enumerate(AC)]
            mlpz = [cpool.tile([sz, N_CLASSES], BF16, name=f"mlpz{i}")
                    for i, (off, sz) in enumerate(MZC)]
            c1b = [cpool.tile([sz, 1], F32, name=f"c1b{i}") for i, (_, sz) in enumerate(OC)]
            c2b = [cpool.tile([sz, 1], F32, name=f"c2b{i}") for i, (_, sz) in enumerate(OC)]
            cc1b = [cpool.tile([sz, 1], F32, name=f"cc1b{i}") for i, (_, sz) in enumerate(CC)]
            cc2b = [cpool.tile([sz, 1], F32, name=f"cc2b{i}") for i, (_, sz) in enumerate(CC)]
            ident = cpool.tile([128, 128], F32, name="ident")

            # persistent state
            hm0 = spool.tile([128, NLOC], BF16, name="hm0")
            hm1 = spool.tile([73, NLOC], BF16, name="hm1")   # row 72 = ones
            h8 = spool.tile([128, 2, NLOC], F8, name="h8")   # [., 1, .] row 72 = ones
            a0 = spool.tile([128, NLOC], BF16, name="a0")
            a1 = spool.tile([73, NLOC], BF16, name="a1")     # row 72 = ones
            x16 = spool.tile([IN_DIM, NLOC], BF16, name="x16")
            WV = 4                      # phaseB wave size (graphs)
            r16 = [spool.tile([sz, WV * N], BF16, name=f"r16_{i}")
                   for i, (_, sz) in enumerate(OC)]
            z16 = [spool.tile([sz, WV * N], BF16, name=f"z16_{i}")
                   for i, (_, sz) in enumerate(OC)]
            cT1 = spool.tile([128, NLOC], BF16, name="cT1")
            cT2 = spool.tile([44, NLOC], BF16, name="cT2")

            # ---------------- load order: features -> early weights ->
            # adjacency -> late (head) weights, so the init transposes and
            # the first graph's matmuls aren't starved by the 8.4MB adj load.
            lds = [spool.tile([128, 4 * IN_DIM], F32, name=f"ld{i}")
                   for i in range(8)]
            for i in range(8):
                nc.sync.dma_start(
                    lds[i][:].rearrange("p (c d) -> p c d", c=4),
                    d_feat[i * 512:(i + 1) * 512, :].rearrange(
                        "(c p) d -> p c d", p=128))
            nc.scalar.dma_start(ident[:], d_ident[:])
            for i, (off, sz) in enumerate(AC):
                nc.sync.dma_start(wcat[i][:], d_wcat[off:off + sz, :])
            nc.sync.dma_start(whh8[:], d_whh8[:])
            for i, (off, sz) in enumerate(AC):
                nc.sync.dma_start(wih[i][:], d_wih[off:off + sz, :])
            # ---- init state
            nc.vector.memset(hm0[:], 0.0)
            nc.vector.memset(hm1[:], 1.0)
            nc.vector.memset(hm1[0:72, :], 0.0)
            nc.vector.memset(h8[:], 0.0)
            nc.sync.dma_start(h8[72:73, 1, :], d_ones8[:])
            nc.vector.memset(a1[:], 1.0)
            for i in range(8):
                for c in range(4):
                    pt = ps(IN_DIM, 128)
                    nc.tensor.transpose(
                        pt[:], lds[i][:, c * IN_DIM:(c + 1) * IN_DIM],
                        ident[:])
                    cs = slice((i * 4 + c) * 128, (i * 4 + c + 1) * 128)
                    nc.scalar.copy(hm0[0:IN_DIM, cs], pt[:])
                    nc.vector.tensor_copy(x16[:, cs], pt[:])
            nc.vector.tensor_copy(h8[:, 0, :], hm0[:])
            # head concat staging (x16 is static once written)
            nc.scalar.dma_start(cT1[72:128, :], x16[0:56, :])
            nc.scalar.dma_start(cT2[:], x16[56:100, :])

            # ============ GGNN ============
            with (
                tc.tile_pool(name="adjp", bufs=1) as apool,
                tc.tile_pool(name="htgp", bufs=3) as htgpool,
                tc.tile_pool(name="gtmp", bufs=3) as tpool,
            ):
                # adjacency arrives after features/GGNN weights, before the
                # head weights (which aren't needed for ~1ms)
                adjt = [apool.tile([128, 16, N], F8, name=f"adj{g}")
                        for g in range(G)]
                for g in range(G):
                    nc.sync.dma_start(adjt[g][:], d_adj[g])
                for i, (off, sz) in enumerate(AC):
                    nc.sync.dma_start(mlpy[i][:], d_mlpy[off:off + sz, :])
                for i, (off, sz) in enumerate(OC):
                    nc.sync.dma_start(c1w[i][:], d_c1w[off:off + sz, :])
                    nc.sync.dma_start(c2w[i][:], d_c2w[off:off + sz, :])
                    nc.sync.dma_start(c1b[i][:], d_c1b[off:off + sz, :])
                    nc.sync.dma_start(c2b[i][:], d_c2b[off:off + sz, :])
                for i, (off, sz) in enumerate(CC):
                    nc.sync.dma_start(cc1w[i][:], d_cc1w[off:off + sz, :])
                    nc.sync.dma_start(cc2w[i][:], d_cc2w[off:off + sz, :])
                    nc.sync.dma_start(cc1b[i][:], d_cc1b[off:off + sz, :])
                    nc.sync.dma_start(cc2b[i][:], d_cc2b[off:off + sz, :])
                for i, (off, sz) in enumerate(MZC):
                    nc.sync.dma_start(mlpz[i][:], d_mlpz[off:off + sz, :])

                # ---- steps
                def phaseA_ht(g):
                    # Ht transform (bf16) -> htg fp8 cast (DVE)
                    htg = htgpool.tile([128, 4, NT * OUT], F8,
                                       name="htg", tag="htg")
                    for sc in range(4):
                        ncs = slice((g * 4 + sc) * 128, (g * 4 + sc + 1) * 128)
                        for noff, nsz in ((0, 512), (512, 288)):
                            ph = ps(128, nsz)
                            nc.tensor.matmul(
                                ph[:], hm0[:, ncs],
                                wcat[0][:, noff:noff + nsz],
                                start=True, stop=False)
                            nc.tensor.matmul(
                                ph[:], hm1[:, ncs],
                                wcat[1][:, noff:noff + nsz],
                                start=False, stop=True)
                            nc.vector.tensor_copy(
                                htg[:, sc, noff:noff + nsz], ph[:])
                    return htg

                def phaseA_agg(g, htg):
                    # DoubleRow fp8 aggregation + a16 cast (ACT)
                    js = slice(g * N, (g + 1) * N)
                    for oc, (ooff, osz) in enumerate(OC):
                        pa = ps(osz, N)
                        k = 0
                        for t in range(NT):
                            for qq in range(2):
                                nc.tensor.matmul(
                                    pa[:],
                                    htg[:, 2 * qq:2 * qq + 2,
                                        t * OUT + ooff:t * OUT + ooff + osz],
                                    adjt[g][:, (t * 2 + qq) * 2:
                                            (t * 2 + qq) * 2 + 2, :],
                                    perf_mode=DR,
                                    start=(k == 0), stop=(k == 7))
                                k += 1
                        if oc == 0:
                            nc.scalar.copy(a0[:, js], pa[:])
                        else:
                            nc.scalar.copy(a1[0:72, js], pa[:])

                def phaseB_rz(g):
                    # r/z gate matmuls + sigmoids into wave-local r16/z16
                    js = slice(g * N, (g + 1) * N)
                    jw = slice((g % WV) * N, (g % WV + 1) * N)
                    for gt, goff in ((r16, 0), (z16, OUT)):
                        for mc, (moff, msz) in enumerate(OC):
                            col = slice(goff + moff, goff + moff + msz)
                            pg = ps(msz, N)
                            nc.tensor.matmul(pg[:], wih[0][:, col],
                                             a0[:, js],
                                             start=True, stop=False)
                            nc.tensor.matmul(pg[:], wih[1][:, col],
                                             a1[:, js],
                                             start=False, stop=False)
                            nc.tensor.matmul(pg[:], whh8[:, :, col],
                                             h8[:, :, js], perf_mode=DR,
                                             start=False, stop=True)
                            nc.scalar.activation(
                                gt[mc][0:msz, jw], pg[:],
                                mybir.ActivationFunctionType.Sigmoid)

                def phaseB_n(g, step):
                    # n gate + state update + h8 recast
                    js = slice(g * N, (g + 1) * N)
                    jw = slice((g % WV) * N, (g % WV + 1) * N)
                    nns = []
                    for mc, (moff, msz) in enumerate(OC):
                        col = slice(2 * OUT + moff, 2 * OUT + moff + msz)
                        pin = ps(msz, N)
                        nc.tensor.matmul(pin[:], wih[0][:, col], a0[:, js],
                                         start=True, stop=False)
                        nc.tensor.matmul(pin[:], wih[1][:, col], a1[:, js],
                                         start=False, stop=True)
                        phn = ps(msz, N)
                        nc.tensor.matmul(phn[:], whh8[:, :, col],
                                         h8[:, :, js], perf_mode=DR,
                                         start=True, stop=True)
                        t1 = tpool.tile([msz, N], BF16, name=f"t1_{mc}",
                                        tag=f"t1_{mc}")
                        nc.vector.tensor_mul(t1[:], r16[mc][0:msz, jw], phn[:])
                        nc.vector.tensor_add(t1[:], t1[:], pin[:])
                        nn = tpool.tile([msz, N], BF16, name=f"nn_{mc}",
                                        tag=f"nn_{mc}")
                        nc.scalar.activation(nn[:], t1[:],
                                             mybir.ActivationFunctionType.Tanh)
                        nns.append(nn)
                    # h' = nn + z*(h-nn)
                    for mc, (moff, msz) in enumerate(OC):
                        nn = nns[mc]
                        hs = hm0[:, js] if mc == 0 else hm1[0:72, js]
                        d1 = tpool.tile([msz, N], BF16, name=f"d1_{mc}",
                                        tag=f"d1_{mc}")
                        nc.gpsimd.tensor_sub(d1[:], hs, nn[:])
                        nc.gpsimd.tensor_mul(d1[:], z16[mc][0:msz, jw], d1[:])
                        nc.gpsimd.tensor_add(hs, nn[:], d1[:])
                        if step < N_STEPS - 1:
                            if mc == 0:
                                nc.scalar.copy(h8[:, 0, js], hs)
                            else:
                                nc.scalar.copy(h8[0:72, 1, js], hs)

                for step in range(N_STEPS):
                    # software-pipelined: Ht(g+1) issues before agg(g) so the
                    # PE has work while graph g's htg casts drain; r/z for all
                    # graphs issue before the n/update chains so the PSUM ring
                    # never waits on a lagging DVE/GpSimd consumer.
                    htgs = [None] * G
                    htgs[0] = phaseA_ht(0)
                    for g in range(G):
                        if g + 1 < G:
                            htgs[g + 1] = phaseA_ht(g + 1)
                        phaseA_agg(g, htgs[g])
                        htgs[g] = None
                    for w in range(0, G, WV):
                        for g in range(w, w + WV):
                            phaseB_rz(g)
                        for g in range(w, w + WV):
                            phaseB_n(g, step)

            # ============ head ============
            with (
                tc.tile_pool(name="head", bufs=1) as hpool,
                tc.tile_pool(name="htmp", bufs=2) as htp,
            ):
                L1 = N - 2          # 510
                P1 = 254
                P2 = 127
                y1p = [hpool.tile([sz, G * P1], BF16, name=f"y1p{i}")
                       for i, (_, sz) in enumerate(OC)]
                z1p = [hpool.tile([sz, G * P1], BF16, name=f"z1p{i}")
                       for i, (_, sz) in enumerate(CC)]
                y2p = [hpool.tile([sz, G * P2], BF16, name=f"y2p{i}")
                       for i, (_, sz) in enumerate(AC)]
                z2p = [hpool.tile([sz, G * P2], BF16, name=f"z2p{i}")
                       for i, (_, sz) in enumerate(MZC)]
                avg7 = hpool.tile([N_CLASSES, G], F32, name="avg7")

                # cT1 rows 72:128 (= x rows 0:56) and cT2 were staged at init;
                # fill cT1 rows 0:72 (= final h rows 128:200) per graph on
                # GpSimd so the DVE FIFO isn't head-of-line blocked.
                nc.vector.memset(y2p[1][:], 1.0)
                nc.vector.memset(z2p[2][:], 1.0)
                for g in range(G):
                    js = slice(g * N, (g + 1) * N)
                    nc.vector.tensor_copy(cT1[0:72, js], hm1[0:72, js])
                yin = [hm0, hm1]
                yinsz = [128, 72]
                zin = [hm0, cT1, cT2]
                zinsz = [128, 128, 44]

                def branch(nco, cin, cinsz, w1, b1, w2, b2, p1t, p2t, och, pfx):
                    for g in range(G):
                        for co, (cooff, cosz) in enumerate(och):
                            pc = ps(cosz, L1)
                            kk = 0
                            nk = 3 * len(cin)
                            for k in range(3):
                                for ci in range(len(cin)):
                                    nc.tensor.matmul(
                                        pc[:],
                                        w1[ci][:, k * nco + cooff:
                                               k * nco + cooff + cosz],
                                        cin[ci][0:cinsz[ci],
                                                g * N + k:g * N + k + L1],
                                        start=(kk == 0), stop=(kk == nk - 1))
                                    kk += 1
                            rl = htp.tile([cosz, L1], BF16, name=f"rl{pfx}{co}",
                                          tag=f"rl{pfx}{co}")
                            nc.scalar.activation(
                                rl[:], pc[:],
                                mybir.ActivationFunctionType.Relu,
                                bias=b1[co][:])
                            m1 = htp.tile([cosz, P1], BF16, name=f"m1{pfx}{co}",
                                          tag=f"m1{pfx}{co}")
                            nc.vector.tensor_max(m1[:], rl[:, 0:508:2],
                                                 rl[:, 1:509:2])
                            nc.vector.tensor_max(
                                p1t[co][0:cosz, g * P1:(g + 1) * P1],
                                m1[:], rl[:, 2:510:2])
                    for g in range(G):
                        for co, (cooff, cosz) in enumerate(och):
                            pc = ps(cosz, P1)
                            for ci, (cioff, cisz) in enumerate(och):
                                nc.tensor.matmul(
                                    pc[:],
                                    w2[ci][:, cooff:cooff + cosz],
                                    p1t[ci][0:cisz, g * P1:(g + 1) * P1],
                                    start=(ci == 0), stop=(ci == len(och) - 1))
                            rl = htp.tile([cosz, P1], BF16, name=f"rw{pfx}{co}",
                                          tag=f"rw{pfx}{co}")
                            nc.scalar.activation(
                                rl[:], pc[:],
                                mybir.ActivationFunctionType.Relu,
                                bias=b2[co][:])
                            nc.vector.tensor_max(
                                p2t[co][0:cosz, g * P2:(g + 1) * P2],
                                rl[:, 0:254:2], rl[:, 1:254:2])

                branch(OUT, yin, yinsz, c1w, c1b, c2w, c2b, y1p, y2p, OC, "y")
                branch(CONCAT, zin, zinsz, cc1w, cc1b, cc2w, cc2b, z1p, z2p,
                       CC, "z")

                # mlps + product + mean over positions
                for g in range(G):
                    gs = slice(g * P2, (g + 1) * P2)
                    pby = ps(N_CLASSES, P2)
                    for k, (off, sz) in enumerate(AC):
                        nc.tensor.matmul(pby[:], mlpy[k][:], y2p[k][0:sz, gs],
                                         start=(k == 0), stop=(k == 1))
                    pbz = ps(N_CLASSES, P2)
                    for k, (off, sz) in enumerate(MZC):
                        nc.tensor.matmul(pbz[:], mlpz[k][:], z2p[k][0:sz, gs],
                                         start=(k == 0), stop=(k == 2))
                    t1 = htp.tile([N_CLASSES, P2], F32, name="tm1", tag="tm1")
                    nc.vector.tensor_copy(t1[:], pby[:])
                    nc.vector.tensor_mul(t1[:], t1[:], pbz[:])
                    nc.vector.reduce_sum(avg7[:, g:g + 1], t1[:],
                                         axis=mybir.AxisListType.X)

                # transpose [7, G] -> [G, 7], softmax rows, write out
                pT = ps(G, N_CLASSES)
                nc.tensor.transpose(pT[:], avg7[:],
                                    ident[0:N_CLASSES, 0:N_CLASSES])
                sm = htp.tile([G, N_CLASSES], F32, name="sm", tag="sm")
                nc.vector.tensor_copy(sm[:], pT[:])
                mx = htp.tile([G, 1], F32, name="mx", tag="mx")
                nc.vector.reduce_max(mx[:], sm[:], axis=mybir.AxisListType.X)
                nmx = htp.tile([G, 1], F32, name="nmx", tag="nmx")
                nc.vector.tensor_scalar_mul(nmx[:], mx[:], -1.0)
                ex = htp.tile([G, N_CLASSES], F32, name="ex", tag="ex")
                ssum = htp.tile([G, 1], F32, name="ssum", tag="ssum")
                nc.scalar.activation(ex[:], sm[:],
                                     mybir.ActivationFunctionType.Exp,
                                     bias=nmx[:], accum_out=ssum[:])
                rec = htp.tile([G, 1], F32, name="rec", tag="rec")
                nc.vector.reciprocal(rec[:], ssum[:])
                fin = htp.tile([G, N_CLASSES], F32, name="fin", tag="fin")
                nc.vector.tensor_scalar_mul(fin[:], ex[:], rec[:])
                nc.sync.dma_start(d_out[:], fin[:])

    nc.compile()
    return nc


_NC = None


def _get_nc():
    global _NC
    if _NC is None:
        _NC = _build_program()
    return _NC


def _prep_inputs(features, edge_src, edge_dst, edge_types, etype_W, etype_b,
                 W_ih, b_ih, W_hh, b_hh, conv1_w, conv1_b, conv2_w, conv2_b,
                 convc1_w, convc1_b, convc2_w, convc2_b,
                 mlp_y_w, mlp_y_b, mlp_z_w, mlp_z_b):
    bf = ml_dtypes.bfloat16
    f8 = ml_dtypes.float8_e4m3
    f32 = np.float32

    # dense adjacency counts: adj[graph, src%128, t*4 + src//128, dst]
    gid = (edge_src // N).astype(np.int64)
    s = (edge_src % N).astype(np.int64)
    d = (edge_dst % N).astype(np.int64)
    t = edge_types.astype(np.int64)
    adj = np.zeros((B, 128, 16, N), np.float32)
    np.add.at(adj, (gid, s % 128, t * 4 + s // 128, d), 1.0)
    adj = adj.astype(f8)

    wcat = np.zeros((OUT + 1, NT * OUT), f32)
    for tt in range(NT):
        wcat[:OUT, tt * OUT:(tt + 1) * OUT] = etype_W[tt].T
        wcat[OUT, tt * OUT:(tt + 1) * OUT] = etype_b[tt]
    wih = np.concatenate([W_ih.T, b_ih[None, :]], 0)
    whhT = np.concatenate([W_hh.T, b_hh[None, :]], 0)   # [201, 600]
    whh8 = np.zeros((128, 2, WPAD), f32)
    whh8[:, 0, 0:3 * OUT] = whhT[0:128]
    whh8[0:73, 1, 0:3 * OUT] = whhT[128:201]
    c1w = conv1_w.transpose(1, 2, 0).reshape(OUT, 3 * OUT)
    c2w = conv2_w[:, :, 0].T
    cc1w = convc1_w.transpose(1, 2, 0).reshape(CONCAT, 3 * CONCAT)
    cc2w = convc2_w[:, :, 0].T
    s127 = np.float32(1.0 / np.sqrt(127.0))
    mlpy = np.concatenate([mlp_y_w.T, mlp_y_b[None, :]], 0) * s127
    mlpz = np.concatenate([mlp_z_w.T, mlp_z_b[None, :]], 0) * s127
    ident = np.eye(128, dtype=f32)

    shared = dict(
        wcat=wcat.astype(bf), wih=wih.astype(bf), whh8=whh8.astype(f8),
        c1w=c1w.astype(bf), c2w=c2w.astype(bf),
        cc1w=cc1w.astype(bf), cc2w=cc2w.astype(bf),
        mlpy=mlpy.astype(bf), mlpz=mlpz.astype(bf),
        c1b=conv1_b.reshape(-1, 1).astype(f32),
        c2b=conv2_b.reshape(-1, 1).astype(f32),
        cc1b=convc1_b.reshape(-1, 1).astype(f32),
        cc2b=convc2_b.reshape(-1, 1).astype(f32),
        ident=ident,
        ones8=np.ones((1, NLOC), np.float32).astype(f8),
    )
    in_maps = []
    for c in range(N_CORES):
        m = dict(shared)
        m["features"] = np.ascontiguousarray(features[c * NLOC:(c + 1) * NLOC]).astype(f32)
        m["adj"] = np.ascontiguousarray(adj[c * G:(c + 1) * G])
        in_maps.append(m)
    return in_maps


def run(inputs, trace=False):
    nc = _get_nc()
    in_maps = _prep_inputs(**inputs)
    res = run_bass_kernel_spmd(nc, in_maps, core_ids=list(range(N_CORES)),
                               trace=trace)
    out = np.concatenate([res.results[i]["out"] for i in range(N_CORES)], 0)
    return out.astype(np.float32), res


def kernel(**inputs):
    out, _ = run(inputs, trace=False)
    return out


# revision 7
# speedup vs baseline: 1.2241x; 1.2241x over previous
"""Devign GGNN + conv-head kernel for 8 Trainium2 NeuronCores — v2.

Changes vs baseline:
- Adjacency stored fp8-e4m3 (counts <= 3, exact) and SBUF-resident across
  all 8 GGNN steps (was: bf16 re-streamed every step, 134 MB DMA).
- Aggregation matmuls use fp8 DoubleRow (K=256 per instruction): stationary
  Ht tiles are cast to fp8, moving adjacency is fp8.  16 -> 8 matmuls per
  (graph, out-chunk).
- GRU h-side gate matmuls (gh = h @ W_hh^T) use fp8 DoubleRow with a paired
  [128, 2, *] layout for h and W_hh^T.  Weight-side fp8 for wcat/wih is NOT
  used (systematic quantization error too large; measured in numpy sim).
- h master state in bf16 (was f32), GRU elementwise update moved to GpSimd,
  PSUM->SBUF casts split between Vector and Scalar engines.
"""

import numpy as np
import ml_dtypes

import concourse.bass as bass
import concourse.bacc as bacc
import concourse.tile as tile
from concourse import mybir
from concourse._compat import get_trn_type
from concourse.bass_utils import run_bass_kernel_spmd

BF16 = mybir.dt.bfloat16
F32 = mybir.dt.float32
F8 = mybir.dt.float8e4
DR = mybir.MatmulPerfMode.DoubleRow

N_CORES = 8
B = 64                    # total graphs
G = B // N_CORES          # graphs per core = 8
N = 512                   # nodes per graph
NLOC = G * N              # nodes per core = 4096
IN_DIM = 100
OUT = 200                 # hidden dim
NT = 4                    # edge types
N_STEPS = 8
N_CLASSES = 7
CONCAT = IN_DIM + OUT     # 300

OC = [(0, 128), (128, 72)]          # plain 200 rows
AC = [(0, 128), (128, 73)]          # 201 rows incl ones row
CC = [(0, 128), (128, 128), (256, 44)]   # 300 rows (concat branch)
MZC = [(0, 128), (128, 128), (256, 45)]  # 301 rows incl ones
WPAD = 608                # padded col count for whh8 (608 % 16 == 0)


def _build_program():
    nc = bacc.Bacc(get_trn_type() or "TRN2", target_bir_lowering=False)

    # ---------------- DRAM I/O ----------------
    d_feat = nc.dram_tensor("features", (NLOC, IN_DIM), F32, kind="ExternalInput")
    d_adj = nc.dram_tensor("adj", (G, 128, 16, N), F8, kind="ExternalInput")
    d_wcat = nc.dram_tensor("wcat", (OUT + 1, NT * OUT), BF16, kind="ExternalInput")
    d_wih = nc.dram_tensor("wih", (OUT + 1, 3 * OUT), BF16, kind="ExternalInput")
    d_whh8 = nc.dram_tensor("whh8", (128, 2, WPAD), F8, kind="ExternalInput")
    d_c1w = nc.dram_tensor("c1w", (OUT, 3 * OUT), BF16, kind="ExternalInput")
    d_c2w = nc.dram_tensor("c2w", (OUT, OUT), BF16, kind="ExternalInput")
    d_cc1w = nc.dram_tensor("cc1w", (CONCAT, 3 * CONCAT), BF16, kind="ExternalInput")
    d_cc2w = nc.dram_tensor("cc2w", (CONCAT, CONCAT), BF16, kind="ExternalInput")
    d_mlpy = nc.dram_tensor("mlpy", (OUT + 1, N_CLASSES), BF16, kind="ExternalInput")
    d_mlpz = nc.dram_tensor("mlpz", (CONCAT + 1, N_CLASSES), BF16, kind="ExternalInput")
    d_c1b = nc.dram_tensor("c1b", (OUT, 1), F32, kind="ExternalInput")
    d_c2b = nc.dram_tensor("c2b", (OUT, 1), F32, kind="ExternalInput")
    d_cc1b = nc.dram_tensor("cc1b", (CONCAT, 1), F32, kind="ExternalInput")
    d_cc2b = nc.dram_tensor("cc2b", (CONCAT, 1), F32, kind="ExternalInput")
    d_ident = nc.dram_tensor("ident", (128, 128), F32, kind="ExternalInput")
    d_ones8 = nc.dram_tensor("ones8", (1, NLOC), F8, kind="ExternalInput")
    d_out = nc.dram_tensor("out", (G, N_CLASSES), F32, kind="ExternalOutput")

    with tile.TileContext(nc) as tc:
        with (
            tc.tile_pool(name="const", bufs=1) as cpool,
            tc.tile_pool(name="state", bufs=1) as spool,
            tc.tile_pool(name="psall", bufs=8, space="PSUM") as pspool,
        ):
            def ps(p_, n_):
                return pspool.tile([p_, n_], F32, name="ps", tag="ps")

            # ---------------- persistent SBUF tensors ----------------
            wcat = [cpool.tile([sz, NT * OUT], BF16, name=f"wcat{i}")
                    for i, (off, sz) in enumerate(AC)]
            wih = [cpool.tile([sz, 3 * OUT], BF16, name=f"wih{i}")
                   for i, (off, sz) in enumerate(AC)]
            whh8 = cpool.tile([128, 2, WPAD], F8, name="whh8")
            c1w = [cpool.tile([sz, 3 * OUT], BF16, name=f"c1w{i}")
                   for i, (off, sz) in enumerate(OC)]
            c2w = [cpool.tile([sz, OUT], BF16, name=f"c2w{i}")
                   for i, (off, sz) in enumerate(OC)]
            cc1w = [cpool.tile([sz, 3 * CONCAT], BF16, name=f"cc1w{i}")
                    for i, (off, sz) in enumerate(CC)]
            cc2w = [cpool.tile([sz, CONCAT], BF16, name=f"cc2w{i}")
                    for i, (off, sz) in enumerate(CC)]
            mlpy = [cpool.tile([sz, N_CLASSES], BF16, name=f"mlpy{i}")
                    for i, (off, sz) in enumerate(AC)]
            mlpz = [cpool.tile([sz, N_CLASSES], BF16, name=f"mlpz{i}")
                    for i, (off, sz) in enumerate(MZC)]
            c1b = [cpool.tile([sz, 1], F32, name=f"c1b{i}") for i, (_, sz) in enumerate(OC)]
            c2b = [cpool.tile([sz, 1], F32, name=f"c2b{i}") for i, (_, sz) in enumerate(OC)]
            cc1b = [cpool.tile([sz, 1], F32, name=f"cc1b{i}") for i, (_, sz) in enumerate(CC)]
            cc2b = [cpool.tile([sz, 1], F32, name=f"cc2b{i}") for i, (_, sz) in enumerate(CC)]
            ident = cpool.tile([128, 128], F32, name="ident")

            # persistent state
            hm0 = spool.tile([128, NLOC], BF16, name="hm0")
            hm1 = spool.tile([73, NLOC], BF16, name="hm1")   # row 72 = ones
            h8 = spool.tile([128, 2, NLOC], F8, name="h8")   # [., 1, .] row 72 = ones
            a0 = spool.tile([128, NLOC], BF16, name="a0")
            a1 = spool.tile([73, NLOC], BF16, name="a1")     # row 72 = ones
            x16 = spool.tile([IN_DIM, NLOC], BF16, name="x16")
            cT1 = spool.tile([128, NLOC], BF16, name="cT1")
            cT2 = spool.tile([44, NLOC], BF16, name="cT2")

            # ---------------- load order: features -> early weights ->
            # adjacency -> late (head) weights, so the init transposes and
            # the first graph's matmuls aren't starved by the 8.4MB adj load.
            lds = [spool.tile([128, 4 * IN_DIM], F32, name=f"ld{i}")
                   for i in range(8)]
            for i in range(8):
                nc.sync.dma_start(
                    lds[i][:].rearrange("p (c d) -> p c d", c=4),
                    d_feat[i * 512:(i + 1) * 512, :].rearrange(
                        "(c p) d -> p c d", p=128))
            nc.scalar.dma_start(ident[:], d_ident[:])
            for i, (off, sz) in enumerate(AC):
                nc.sync.dma_start(wcat[i][:], d_wcat[off:off + sz, :])
            nc.sync.dma_start(whh8[:], d_whh8[:])
            for i, (off, sz) in enumerate(AC):
                nc.sync.dma_start(wih[i][:], d_wih[off:off + sz, :])
            # ---- init state
            nc.vector.memset(hm0[:], 0.0)
            nc.vector.memset(hm1[:], 1.0)
            nc.vector.memset(hm1[0:72, :], 0.0)
            nc.vector.memset(h8[:], 0.0)
            nc.sync.dma_start(h8[72:73, 1, :], d_ones8[:])
            nc.vector.memset(a1[:], 1.0)
            for i in range(8):
                for c in range(4):
                    pt = ps(IN_DIM, 128)
                    nc.tensor.transpose(
                        pt[:], lds[i][:, c * IN_DIM:(c + 1) * IN_DIM],
                        ident[:])
                    cs = slice((i * 4 + c) * 128, (i * 4 + c + 1) * 128)
                    nc.scalar.copy(hm0[0:IN_DIM, cs], pt[:])
                    nc.vector.tensor_copy(x16[:, cs], pt[:])
            nc.vector.tensor_copy(h8[:, 0, :], hm0[:])
            # head concat staging (x16 is static once written)
            nc.scalar.dma_start(cT1[72:128, :], x16[0:56, :])
            nc.scalar.dma_start(cT2[:], x16[56:100, :])

            # ============ GGNN ============
            with (
                tc.tile_pool(name="adjp", bufs=1) as apool,
                tc.tile_pool(name="htgp", bufs=3) as htgpool,
                tc.tile_pool(name="gtmp", bufs=3) as tpool,
            ):
                # adjacency arrives after features/GGNN weights, before the
                # head weights (which aren't needed for ~1ms)
                adjt = [apool.tile([128, 16, N], F8, name=f"adj{g}")
                        for g in range(G)]
                for g in range(G):
                    nc.sync.dma_start(adjt[g][:], d_adj[g])
                for i, (off, sz) in enumerate(AC):
                    nc.sync.dma_start(mlpy[i][:], d_mlpy[off:off + sz, :])
                for i, (off, sz) in enumerate(OC):
                    nc.sync.dma_start(c1w[i][:], d_c1w[off:off + sz, :])
                    nc.sync.dma_start(c2w[i][:], d_c2w[off:off + sz, :])
                    nc.sync.dma_start(c1b[i][:], d_c1b[off:off + sz, :])
                    nc.sync.dma_start(c2b[i][:], d_c2b[off:off + sz, :])
                for i, (off, sz) in enumerate(CC):
                    nc.sync.dma_start(cc1w[i][:], d_cc1w[off:off + sz, :])
                    nc.sync.dma_start(cc2w[i][:], d_cc2w[off:off + sz, :])
                    nc.sync.dma_start(cc1b[i][:], d_cc1b[off:off + sz, :])
                    nc.sync.dma_start(cc2b[i][:], d_cc2b[off:off + sz, :])
                for i, (off, sz) in enumerate(MZC):
                    nc.sync.dma_start(mlpz[i][:], d_mlpz[off:off + sz, :])

                # ---- steps
                def phaseA_ht(g):
                    # Ht transform (bf16) -> htg fp8 cast (DVE)
                    htg = htgpool.tile([128, 4, NT * OUT], F8,
                                       name="htg", tag="htg")
                    for sc in range(4):
                        ncs = slice((g * 4 + sc) * 128, (g * 4 + sc + 1) * 128)
                        for noff, nsz in ((0, 512), (512, 288)):
                            ph = ps(128, nsz)
                            nc.tensor.matmul(
                                ph[:], hm0[:, ncs],
                                wcat[0][:, noff:noff + nsz],
                                start=True, stop=False)
                            nc.tensor.matmul(
                                ph[:], hm1[:, ncs],
                                wcat[1][:, noff:noff + nsz],
                                start=False, stop=True)
                            nc.vector.tensor_copy(
                                htg[:, sc, noff:noff + nsz], ph[:])
                    return htg

                def phaseA_agg(g, htg):
                    # DoubleRow fp8 aggregation + a16 cast (ACT)
                    js = slice(g * N, (g + 1) * N)
                    for oc, (ooff, osz) in enumerate(OC):
                        pa = ps(osz, N)
                        k = 0
                        for t in range(NT):
                            for qq in range(2):
                                nc.tensor.matmul(
                                    pa[:],
                                    htg[:, 2 * qq:2 * qq + 2,
                                        t * OUT + ooff:t * OUT + ooff + osz],
                                    adjt[g][:, (t * 2 + qq) * 2:
                                            (t * 2 + qq) * 2 + 2, :],
                                    perf_mode=DR,
                                    start=(k == 0), stop=(k == 7))
                                k += 1
                        if oc == 0:
                            nc.scalar.copy(a0[:, js], pa[:])
                        else:
                            nc.scalar.copy(a1[0:72, js], pa[:])

                def phaseB(g, step):
                    # gates + GRU update + h8 recast for one graph
                    js = slice(g * N, (g + 1) * N)
                    rz = []
                    for gname, goff in (("r", 0), ("z", OUT)):
                        gt = []
                        for mc, (moff, msz) in enumerate(OC):
                            col = slice(goff + moff, goff + moff + msz)
                            pg = ps(msz, N)
                            nc.tensor.matmul(pg[:], wih[0][:, col],
                                             a0[:, js],
                                             start=True, stop=False)
                            nc.tensor.matmul(pg[:], wih[1][:, col],
                                             a1[:, js],
                                             start=False, stop=False)
                            nc.tensor.matmul(pg[:], whh8[:, :, col],
                                             h8[:, :, js], perf_mode=DR,
                                             start=False, stop=True)
                            st = tpool.tile([msz, N], BF16,
                                            name=f"{gname}{mc}",
                                            tag=f"{gname}{mc}")
                            nc.scalar.activation(
                                st[:], pg[:],
                                mybir.ActivationFunctionType.Sigmoid)
                            gt.append(st)
                        rz.append(gt)
                    r16g, z16g = rz

                    nns = []
                    for mc, (moff, msz) in enumerate(OC):
                        col = slice(2 * OUT + moff, 2 * OUT + moff + msz)
                        pin = ps(msz, N)
                        nc.tensor.matmul(pin[:], wih[0][:, col], a0[:, js],
                                         start=True, stop=False)
                        nc.tensor.matmul(pin[:], wih[1][:, col], a1[:, js],
                                         start=False, stop=True)
                        phn = ps(msz, N)
                        nc.tensor.matmul(phn[:], whh8[:, :, col],
                                         h8[:, :, js], perf_mode=DR,
                                         start=True, stop=True)
                        t1 = tpool.tile([msz, N], BF16, name=f"t1_{mc}",
                                        tag=f"t1_{mc}")
                        nc.vector.tensor_mul(t1[:], r16g[mc][:], phn[:])
                        nc.vector.tensor_add(t1[:], t1[:], pin[:])
                        nn = tpool.tile([msz, N], BF16, name=f"nn_{mc}",
                                        tag=f"nn_{mc}")
                        nc.scalar.activation(nn[:], t1[:],
                                             mybir.ActivationFunctionType.Tanh)
                        nns.append(nn)
                    # h' = nn + z*(h-nn); sub on DVE (bf16 2x), rest GpSimd
                    for mc, (moff, msz) in enumerate(OC):
                        nn = nns[mc]
                        hs = hm0[:, js] if mc == 0 else hm1[0:72, js]
                        d1 = tpool.tile([msz, N], BF16, name=f"d1_{mc}",
                                        tag=f"d1_{mc}")
                        nc.vector.tensor_sub(d1[:], hs, nn[:])
                        nc.gpsimd.tensor_mul(d1[:], z16g[mc][:], d1[:])
                        nc.gpsimd.tensor_add(hs, nn[:], d1[:])
                        if step < N_STEPS - 1:
                            if mc == 0:
                                nc.vector.tensor_copy(h8[:, 0, js], hs)
                            else:
                                nc.vector.tensor_copy(h8[0:72, 1, js], hs)

                # Cross-step software pipeline at graph granularity: while
                # step s-1's per-graph gate/update chains drain on
                # ACT/DVE/GpSimd, the PE runs step s's Ht/aggregation for
                # graphs two behind.  s ranges over 0..N_STEPS; iteration s
                # issues phaseB(s-1, *) interleaved with phaseA(s, *).
                htgs = [None] * G
                for s in range(N_STEPS + 1):
                    if s == 0:
                        htgs[0] = phaseA_ht(0)
                        for g in range(G):
                            if g + 1 < G:
                                htgs[g + 1] = phaseA_ht(g + 1)
                            phaseA_agg(g, htgs[g])
                    elif s < N_STEPS:
                        for g in range(G):
                            phaseB(g, s - 1)
                            if g >= 2:
                                htgs[g - 2] = phaseA_ht(g - 2)
                            if g >= 3:
                                phaseA_agg(g - 3, htgs[g - 3])
                        htgs[G - 2] = phaseA_ht(G - 2)
                        phaseA_agg(G - 3, htgs[G - 3])
                        htgs[G - 1] = phaseA_ht(G - 1)
                        phaseA_agg(G - 2, htgs[G - 2])
                        phaseA_agg(G - 1, htgs[G - 1])
                    else:
                        for g in range(G):
                            phaseB(g, N_STEPS - 1)

            # ============ head ============
            with (
                tc.tile_pool(name="head", bufs=1) as hpool,
                tc.tile_pool(name="htmp", bufs=2) as htp,
            ):
                L1 = N - 2          # 510
                P1 = 254
                P2 = 127
                y1p = [hpool.tile([sz, G * P1], BF16, name=f"y1p{i}")
                       for i, (_, sz) in enumerate(OC)]
                z1p = [hpool.tile([sz, G * P1], BF16, name=f"z1p{i}")
                       for i, (_, sz) in enumerate(CC)]
                y2p = [hpool.tile([sz, G * P2], BF16, name=f"y2p{i}")
                       for i, (_, sz) in enumerate(AC)]
                z2p = [hpool.tile([sz, G * P2], BF16, name=f"z2p{i}")
                       for i, (_, sz) in enumerate(MZC)]
                avg7 = hpool.tile([N_CLASSES, G], F32, name="avg7")

                # cT1 rows 72:128 (= x rows 0:56) and cT2 were staged at init;
                # fill cT1 rows 0:72 (= final h rows 128:200) per graph on
                # GpSimd so the DVE FIFO isn't head-of-line blocked.
                nc.vector.memset(y2p[1][:], 1.0)
                nc.vector.memset(z2p[2][:], 1.0)
                for g in range(G):
                    js = slice(g * N, (g + 1) * N)
                    nc.vector.tensor_copy(cT1[0:72, js], hm1[0:72, js])
                yin = [hm0, hm1]
                yinsz = [128, 72]
                zin = [hm0, cT1, cT2]
                zinsz = [128, 128, 44]

                def branch(nco, cin, cinsz, w1, b1, w2, b2, p1t, p2t, och, pfx):
                    for g in range(G):
                        for co, (cooff, cosz) in enumerate(och):
                            pc = ps(cosz, L1)
                            kk = 0
                            nk = 3 * len(cin)
                            for k in range(3):
                                for ci in range(len(cin)):
                                    nc.tensor.matmul(
                                        pc[:],
                                        w1[ci][:, k * nco + cooff:
                                               k * nco + cooff + cosz],
                                        cin[ci][0:cinsz[ci],
                                                g * N + k:g * N + k + L1],
                                        start=(kk == 0), stop=(kk == nk - 1))
                                    kk += 1
                            rl = htp.tile([cosz, L1], BF16, name=f"rl{pfx}{co}",
                                          tag=f"rl{pfx}{co}")
                            nc.scalar.activation(
                                rl[:], pc[:],
                                mybir.ActivationFunctionType.Relu,
                                bias=b1[co][:])
                            m1 = htp.tile([cosz, P1], BF16, name=f"m1{pfx}{co}",
                                          tag=f"m1{pfx}{co}")
                            nc.vector.tensor_max(m1[:], rl[:, 0:508:2],
                                                 rl[:, 1:509:2])
                            nc.vector.tensor_max(
                                p1t[co][0:cosz, g * P1:(g + 1) * P1],
                                m1[:], rl[:, 2:510:2])
                    for g in range(G):
                        for co, (cooff, cosz) in enumerate(och):
                            pc = ps(cosz, P1)
                            for ci, (cioff, cisz) in enumerate(och):
                                nc.tensor.matmul(
                                    pc[:],
                                    w2[ci][:, cooff:cooff + cosz],
                                    p1t[ci][0:cisz, g * P1:(g + 1) * P1],
                                    start=(ci == 0), stop=(ci == len(och) - 1))
                            rl = htp.tile([cosz, P1], BF16, name=f"rw{pfx}{co}",
                                          tag=f"rw{pfx}{co}")
                            nc.scalar.activation(
                                rl[:], pc[:],
                                mybir.ActivationFunctionType.Relu,
                                bias=b2[co][:])
                            nc.vector.tensor_max(
                                p2t[co][0:cosz, g * P2:(g + 1) * P2],
                                rl[:, 0:254:2], rl[:, 1:254:2])

                branch(OUT, yin, yinsz, c1w, c1b, c2w, c2b, y1p, y2p, OC, "y")
                branch(CONCAT, zin, zinsz, cc1w, cc1b, cc2w, cc2b, z1p, z2p,
                       CC, "z")

                # mlps + product + mean over positions
                for g in range(G):
                    gs = slice(g * P2, (g + 1) * P2)
                    pby = ps(N_CLASSES, P2)
                    for k, (off, sz) in enumerate(AC):
                        nc.tensor.matmul(pby[:], mlpy[k][:], y2p[k][0:sz, gs],
                                         start=(k == 0), stop=(k == 1))
                    pbz = ps(N_CLASSES, P2)
                    for k, (off, sz) in enumerate(MZC):
                        nc.tensor.matmul(pbz[:], mlpz[k][:], z2p[k][0:sz, gs],
                                         start=(k == 0), stop=(k == 2))
                    t1 = htp.tile([N_CLASSES, P2], F32, name="tm1", tag="tm1")
                    nc.vector.tensor_copy(t1[:], pby[:])
                    nc.vector.tensor_mul(t1[:], t1[:], pbz[:])
                    nc.vector.reduce_sum(avg7[:, g:g + 1], t1[:],
                                         axis=mybir.AxisListType.X)

                # transpose [7, G] -> [G, 7], softmax rows, write out
                pT = ps(G, N_CLASSES)
                nc.tensor.transpose(pT[:], avg7[:],
                                    ident[0:N_CLASSES, 0:N_CLASSES])
                sm = htp.tile([G, N_CLASSES], F32, name="sm", tag="sm")
                nc.vector.tensor_copy(sm[:], pT[:])
                mx = htp.tile([G, 1], F32, name="mx", tag="mx")
                nc.vector.reduce_max(mx[:], sm[:], axis=mybir.AxisListType.X)
                nmx = htp.tile([G, 1], F32, name="nmx", tag="nmx")
                nc.vector.tensor_scalar_mul(nmx[:], mx[:], -1.0)
                ex = htp.tile([G, N_CLASSES], F32, name="ex", tag="ex")
                ssum = htp.tile([G, 1], F32, name="ssum", tag="ssum")
                nc.scalar.activation(ex[:], sm[:],
                                     mybir.ActivationFunctionType.Exp,
                                     bias=nmx[:], accum_out=ssum[:])
                rec = htp.tile([G, 1], F32, name="rec", tag="rec")
                nc.vector.reciprocal(rec[:], ssum[:])
                fin = htp.tile([G, N_CLASSES], F32, name="fin", tag="fin")
                nc.vector.tensor_scalar_mul(fin[:], ex[:], rec[:])
                nc.sync.dma_start(d_out[:], fin[:])

    nc.compile()
    return nc


_NC = None


def _get_nc():
    global _NC
    if _NC is None:
        _NC = _build_program()
    return _NC


def _prep_inputs(features, edge_src, edge_dst, edge_types, etype_W, etype_b,
                 W_ih, b_ih, W_hh, b_hh, conv1_w, conv1_b, conv2_w, conv2_b,
                 convc1_w, convc1_b, convc2_w, convc2_b,
                 mlp_y_w, mlp_y_b, mlp_z_w, mlp_z_b):
    bf = ml_dtypes.bfloat16
    f8 = ml_dtypes.float8_e4m3
    f32 = np.float32

    # dense adjacency counts: adj[graph, src%128, t*4 + src//128, dst]
    gid = (edge_src // N).astype(np.int64)
    s = (edge_src % N).astype(np.int64)
    d = (edge_dst % N).astype(np.int64)
    t = edge_types.astype(np.int64)
    adj = np.zeros((B, 128, 16, N), np.float32)
    np.add.at(adj, (gid, s % 128, t * 4 + s // 128, d), 1.0)
    adj = adj.astype(f8)

    wcat = np.zeros((OUT + 1, NT * OUT), f32)
    for tt in range(NT):
        wcat[:OUT, tt * OUT:(tt + 1) * OUT] = etype_W[tt].T
        wcat[OUT, tt * OUT:(tt + 1) * OUT] = etype_b[tt]
    wih = np.concatenate([W_ih.T, b_ih[None, :]], 0)
    whhT = np.concatenate([W_hh.T, b_hh[None, :]], 0)   # [201, 600]
    whh8 = np.zeros((128, 2, WPAD), f32)
    whh8[:, 0, 0:3 * OUT] = whhT[0:128]
    whh8[0:73, 1, 0:3 * OUT] = whhT[128:201]
    c1w = conv1_w.transpose(1, 2, 0).reshape(OUT, 3 * OUT)
    c2w = conv2_w[:, :, 0].T
    cc1w = convc1_w.transpose(1, 2, 0).reshape(CONCAT, 3 * CONCAT)
    cc2w = convc2_w[:, :, 0].T
    s127 = np.float32(1.0 / np.sqrt(127.0))
    mlpy = np.concatenate([mlp_y_w.T, mlp_y_b[None, :]], 0) * s127
    mlpz = np.concatenate([mlp_z_w.T, mlp_z_b[None, :]], 0) * s127
    ident = np.eye(128, dtype=f32)

    shared = dict(
        wcat=wcat.astype(bf), wih=wih.astype(bf), whh8=whh8.astype(f8),
        c1w=c1w.astype(bf), c2w=c2w.astype(bf),
        cc1w=cc1w.astype(bf), cc2w=cc2w.astype(bf),
        mlpy=mlpy.astype(bf), mlpz=mlpz.astype(bf),
        c1b=conv1_b.reshape(-1, 1).astype(f32),
        c2b=conv2_b.reshape(-1, 1).astype(f32),
        cc1b=convc1_b.reshape(-1, 1).astype(f32),
        cc2b=convc2_b.reshape(-1, 1).astype(f32),
        ident=ident,
        ones8=np.ones((1, NLOC), np.float32).astype(f8),
    )
    in_maps = []
    for c in range(N_CORES):
        m = dict(shared)
        m["features"] = np.ascontiguousarray(features[c * NLOC:(c + 1) * NLOC]).astype(f32)
        m["adj"] = np.ascontiguousarray(adj[c * G:(c + 1) * G])
        in_maps.append(m)
    return in_maps


def run(inputs, trace=False):
    nc = _get_nc()
    in_maps = _prep_inputs(**inputs)
    res = run_bass_kernel_spmd(nc, in_maps, core_ids=list(range(N_CORES)),
                               trace=trace)
    out = np.concatenate([res.results[i]["out"] for i in range(N_CORES)], 0)
    return out.astype(np.float32), res


def kernel(**inputs):
    out, _ = run(inputs, trace=False)
    return out


# revision 8
# speedup vs baseline: 1.2795x; 1.0453x over previous
"""Devign GGNN + conv-head kernel for 8 Trainium2 NeuronCores — v2.

Changes vs baseline:
- Adjacency stored fp8-e4m3 (counts <= 3, exact) and SBUF-resident across
  all 8 GGNN steps (was: bf16 re-streamed every step, 134 MB DMA).
- Aggregation matmuls use fp8 DoubleRow (K=256 per instruction): stationary
  Ht tiles are cast to fp8, moving adjacency is fp8.  16 -> 8 matmuls per
  (graph, out-chunk).
- GRU h-side gate matmuls (gh = h @ W_hh^T) use fp8 DoubleRow with a paired
  [128, 2, *] layout for h and W_hh^T.  Weight-side fp8 for wcat/wih is NOT
  used (systematic quantization error too large; measured in numpy sim).
- h master state in bf16 (was f32), GRU elementwise update moved to GpSimd,
  PSUM->SBUF casts split between Vector and Scalar engines.
"""

import numpy as np
import ml_dtypes

import concourse.bass as bass
import concourse.bacc as bacc
import concourse.tile as tile
from concourse import mybir
from concourse._compat import get_trn_type
from concourse.bass_utils import run_bass_kernel_spmd

BF16 = mybir.dt.bfloat16
F32 = mybir.dt.float32
F8 = mybir.dt.float8e4
DR = mybir.MatmulPerfMode.DoubleRow

N_CORES = 8
B = 64                    # total graphs
G = B // N_CORES          # graphs per core = 8
N = 512                   # nodes per graph
NLOC = G * N              # nodes per core = 4096
IN_DIM = 100
OUT = 200                 # hidden dim
NT = 4                    # edge types
N_STEPS = 8
N_CLASSES = 7
CONCAT = IN_DIM + OUT     # 300

OC = [(0, 128), (128, 72)]          # plain 200 rows
AC = [(0, 128), (128, 73)]          # 201 rows incl ones row
CC = [(0, 128), (128, 128), (256, 44)]   # 300 rows (concat branch)
MZC = [(0, 128), (128, 128), (256, 45)]  # 301 rows incl ones
WPAD = 608                # padded col count for whh8 (608 % 16 == 0)


def _build_program():
    nc = bacc.Bacc(get_trn_type() or "TRN2", target_bir_lowering=False)

    # ---------------- DRAM I/O ----------------
    d_feat = nc.dram_tensor("features", (NLOC, IN_DIM), F32, kind="ExternalInput")
    d_adj = nc.dram_tensor("adj", (G, 128, 16, N), F8, kind="ExternalInput")
    d_wcat = nc.dram_tensor("wcat", (OUT + 1, NT * OUT), BF16, kind="ExternalInput")
    d_wih = nc.dram_tensor("wih", (OUT + 1, 3 * OUT), BF16, kind="ExternalInput")
    d_whh8 = nc.dram_tensor("whh8", (128, 2, WPAD), F8, kind="ExternalInput")
    d_c1w = nc.dram_tensor("c1w", (OUT, 3 * OUT), BF16, kind="ExternalInput")
    d_c2w = nc.dram_tensor("c2w", (OUT, OUT), BF16, kind="ExternalInput")
    d_cc1w = nc.dram_tensor("cc1w", (CONCAT, 3 * CONCAT), BF16, kind="ExternalInput")
    d_cc2w = nc.dram_tensor("cc2w", (CONCAT, CONCAT), BF16, kind="ExternalInput")
    d_mlpy = nc.dram_tensor("mlpy", (OUT + 1, N_CLASSES), BF16, kind="ExternalInput")
    d_mlpz = nc.dram_tensor("mlpz", (CONCAT + 1, N_CLASSES), BF16, kind="ExternalInput")
    d_c1b = nc.dram_tensor("c1b", (OUT, 1), F32, kind="ExternalInput")
    d_c2b = nc.dram_tensor("c2b", (OUT, 1), F32, kind="ExternalInput")
    d_cc1b = nc.dram_tensor("cc1b", (CONCAT, 1), F32, kind="ExternalInput")
    d_cc2b = nc.dram_tensor("cc2b", (CONCAT, 1), F32, kind="ExternalInput")
    d_ident = nc.dram_tensor("ident", (128, 128), F32, kind="ExternalInput")
    d_ones8 = nc.dram_tensor("ones8", (1, NLOC), F8, kind="ExternalInput")
    d_out = nc.dram_tensor("out", (G, N_CLASSES), F32, kind="ExternalOutput")

    with tile.TileContext(nc) as tc:
        with (
            tc.tile_pool(name="const", bufs=1) as cpool,
            tc.tile_pool(name="state", bufs=1) as spool,
            tc.tile_pool(name="psall", bufs=8, space="PSUM") as pspool,
        ):
            def ps(p_, n_):
                return pspool.tile([p_, n_], F32, name="ps", tag="ps")

            # ---------------- persistent SBUF tensors ----------------
            wcat = [cpool.tile([sz, NT * OUT], BF16, name=f"wcat{i}")
                    for i, (off, sz) in enumerate(AC)]
            wih = [cpool.tile([sz, 3 * OUT], BF16, name=f"wih{i}")
                   for i, (off, sz) in enumerate(AC)]
            whh8 = cpool.tile([128, 2, WPAD], F8, name="whh8")
            c1w = [cpool.tile([sz, 3 * OUT], BF16, name=f"c1w{i}")
                   for i, (off, sz) in enumerate(OC)]
            c2w = [cpool.tile([sz, OUT], BF16, name=f"c2w{i}")
                   for i, (off, sz) in enumerate(OC)]
            cc1w = [cpool.tile([sz, 3 * CONCAT], BF16, name=f"cc1w{i}")
                    for i, (off, sz) in enumerate(CC)]
            cc2w = [cpool.tile([sz, CONCAT], BF16, name=f"cc2w{i}")
                    for i, (off, sz) in enumerate(CC)]
            mlpy = [cpool.tile([sz, N_CLASSES], BF16, name=f"mlpy{i}")
                    for i, (off, sz) in enumerate(AC)]
            mlpz = [cpool.tile([sz, N_CLASSES], BF16, name=f"mlpz{i}")
                    for i, (off, sz) in enumerate(MZC)]
            c1b = [cpool.tile([sz, 1], F32, name=f"c1b{i}") for i, (_, sz) in enumerate(OC)]
            c2b = [cpool.tile([sz, 1], F32, name=f"c2b{i}") for i, (_, sz) in enumerate(OC)]
            cc1b = [cpool.tile([sz, 1], F32, name=f"cc1b{i}") for i, (_, sz) in enumerate(CC)]
            cc2b = [cpool.tile([sz, 1], F32, name=f"cc2b{i}") for i, (_, sz) in enumerate(CC)]
            ident = cpool.tile([128, 128], F32, name="ident")

            # persistent state
            hm0 = spool.tile([128, NLOC], BF16, name="hm0")
            hm1 = spool.tile([73, NLOC], BF16, name="hm1")   # row 72 = ones
            h8 = spool.tile([128, 2, NLOC], F8, name="h8")   # [., 1, .] row 72 = ones
            a0 = spool.tile([128, NLOC], BF16, name="a0")
            a1 = spool.tile([73, NLOC], BF16, name="a1")     # row 72 = ones
            x16 = spool.tile([IN_DIM, NLOC], BF16, name="x16")
            cT1 = spool.tile([128, NLOC], BF16, name="cT1")
            cT2 = spool.tile([44, NLOC], BF16, name="cT2")

            # ---------------- load order: features -> early weights ->
            # adjacency -> late (head) weights, so the init transposes and
            # the first graph's matmuls aren't starved by the 8.4MB adj load.
            # contiguous feature load: partition p holds rows 32p..32p+31
            ld2 = spool.tile([128, 32 * IN_DIM], F32, name="ld2")
            nc.sync.dma_start(
                ld2[:], d_feat[:].rearrange("(p r) d -> p (r d)", p=128))
            nc.scalar.dma_start(ident[:], d_ident[:])
            for i, (off, sz) in enumerate(AC):
                nc.sync.dma_start(wcat[i][:], d_wcat[off:off + sz, :])
            nc.sync.dma_start(whh8[:], d_whh8[:])
            for i, (off, sz) in enumerate(AC):
                nc.sync.dma_start(wih[i][:], d_wih[off:off + sz, :])
            # ---- init state
            nc.vector.memset(hm0[:], 0.0)
            nc.vector.memset(hm1[:], 1.0)
            nc.vector.memset(hm1[0:72, :], 0.0)
            nc.vector.memset(h8[:], 0.0)
            nc.sync.dma_start(h8[72:73, 1, :], d_ones8[:])
            nc.vector.memset(a1[:], 1.0)
            # transpose chunk c covers nodes {32k+c}: strided column writes
            for c in range(32):
                pt = ps(IN_DIM, 128)
                nc.tensor.transpose(
                    pt[:], ld2[:, c * IN_DIM:(c + 1) * IN_DIM], ident[:])
                nc.scalar.copy(hm0[0:IN_DIM, c:NLOC:32], pt[:])
                nc.vector.tensor_copy(x16[:, c:NLOC:32], pt[:])
            nc.vector.tensor_copy(h8[:, 0, :], hm0[:])
            # head concat staging (x16 is static once written)
            nc.scalar.dma_start(cT1[72:128, :], x16[0:56, :])
            nc.scalar.dma_start(cT2[:], x16[56:100, :])

            # ============ GGNN ============
            with (
                tc.tile_pool(name="adjp", bufs=1) as apool,
                tc.tile_pool(name="htgp", bufs=3) as htgpool,
                tc.tile_pool(name="gtmp", bufs=3) as tpool,
            ):
                # adjacency arrives after features/GGNN weights, before the
                # head weights (which aren't needed for ~1ms)
                adjt = [apool.tile([128, 16, N], F8, name=f"adj{g}")
                        for g in range(G)]
                for g in range(G):
                    nc.sync.dma_start(adjt[g][:], d_adj[g])
                for i, (off, sz) in enumerate(AC):
                    nc.sync.dma_start(mlpy[i][:], d_mlpy[off:off + sz, :])
                for i, (off, sz) in enumerate(OC):
                    nc.sync.dma_start(c1w[i][:], d_c1w[off:off + sz, :])
                    nc.sync.dma_start(c2w[i][:], d_c2w[off:off + sz, :])
                    nc.sync.dma_start(c1b[i][:], d_c1b[off:off + sz, :])
                    nc.sync.dma_start(c2b[i][:], d_c2b[off:off + sz, :])
                for i, (off, sz) in enumerate(CC):
                    nc.sync.dma_start(cc1w[i][:], d_cc1w[off:off + sz, :])
                    nc.sync.dma_start(cc2w[i][:], d_cc2w[off:off + sz, :])
                    nc.sync.dma_start(cc1b[i][:], d_cc1b[off:off + sz, :])
                    nc.sync.dma_start(cc2b[i][:], d_cc2b[off:off + sz, :])
                for i, (off, sz) in enumerate(MZC):
                    nc.sync.dma_start(mlpz[i][:], d_mlpz[off:off + sz, :])

                # ---- steps
                def phaseA_ht(g):
                    # Ht transform (bf16) -> htg fp8 cast (DVE)
                    htg = htgpool.tile([128, 4, NT * OUT], F8,
                                       name="htg", tag="htg")
                    for sc in range(4):
                        ncs = slice((g * 4 + sc) * 128, (g * 4 + sc + 1) * 128)
                        for noff, nsz in ((0, 512), (512, 288)):
                            ph = ps(128, nsz)
                            nc.tensor.matmul(
                                ph[:], hm0[:, ncs],
                                wcat[0][:, noff:noff + nsz],
                                start=True, stop=False)
                            nc.tensor.matmul(
                                ph[:], hm1[:, ncs],
                                wcat[1][:, noff:noff + nsz],
                                start=False, stop=True)
                            nc.vector.tensor_copy(
                                htg[:, sc, noff:noff + nsz], ph[:])
                    return htg

                def phaseA_agg(g, htg):
                    # DoubleRow fp8 aggregation + a16 cast (ACT)
                    js = slice(g * N, (g + 1) * N)
                    for oc, (ooff, osz) in enumerate(OC):
                        pa = ps(osz, N)
                        k = 0
                        for t in range(NT):
                            for qq in range(2):
                                nc.tensor.matmul(
                                    pa[:],
                                    htg[:, 2 * qq:2 * qq + 2,
                                        t * OUT + ooff:t * OUT + ooff + osz],
                                    adjt[g][:, (t * 2 + qq) * 2:
                                            (t * 2 + qq) * 2 + 2, :],
                                    perf_mode=DR,
                                    start=(k == 0), stop=(k == 7))
                                k += 1
                        if oc == 0:
                            nc.scalar.copy(a0[:, js], pa[:])
                        else:
                            nc.scalar.copy(a1[0:72, js], pa[:])

                def phaseB(g, step):
                    # gates + GRU update + h8 recast for one graph
                    js = slice(g * N, (g + 1) * N)
                    rz = []
                    for gname, goff in (("r", 0), ("z", OUT)):
                        gt = []
                        for mc, (moff, msz) in enumerate(OC):
                            col = slice(goff + moff, goff + moff + msz)
                            pg = ps(msz, N)
                            nc.tensor.matmul(pg[:], wih[0][:, col],
                                             a0[:, js],
                                             start=True, stop=False)
                            nc.tensor.matmul(pg[:], wih[1][:, col],
                                             a1[:, js],
                                             start=False, stop=False)
                            nc.tensor.matmul(pg[:], whh8[:, :, col],
                                             h8[:, :, js], perf_mode=DR,
                                             start=False, stop=True)
                            st = tpool.tile([msz, N], BF16,
                                            name=f"{gname}{mc}",
                                            tag=f"{gname}{mc}")
                            nc.scalar.activation(
                                st[:], pg[:],
                                mybir.ActivationFunctionType.Sigmoid)
                            gt.append(st)
                        rz.append(gt)
                    r16g, z16g = rz

                    nns = []
                    for mc, (moff, msz) in enumerate(OC):
                        col = slice(2 * OUT + moff, 2 * OUT + moff + msz)
                        pin = ps(msz, N)
                        nc.tensor.matmul(pin[:], wih[0][:, col], a0[:, js],
                                         start=True, stop=False)
                        nc.tensor.matmul(pin[:], wih[1][:, col], a1[:, js],
                                         start=False, stop=True)
                        phn = ps(msz, N)
                        nc.tensor.matmul(phn[:], whh8[:, :, col],
                                         h8[:, :, js], perf_mode=DR,
                                         start=True, stop=True)
                        t1 = tpool.tile([msz, N], BF16, name=f"t1_{mc}",
                                        tag=f"t1_{mc}")
                        nc.vector.tensor_mul(t1[:], r16g[mc][:], phn[:])
                        nc.vector.tensor_add(t1[:], t1[:], pin[:])
                        nn = tpool.tile([msz, N], BF16, name=f"nn_{mc}",
                                        tag=f"nn_{mc}")
                        nc.scalar.activation(nn[:], t1[:],
                                             mybir.ActivationFunctionType.Tanh)
                        nns.append(nn)
                    # h' = nn + z*(h-nn); sub on DVE (bf16 2x), rest GpSimd
                    for mc, (moff, msz) in enumerate(OC):
                        nn = nns[mc]
                        hs = hm0[:, js] if mc == 0 else hm1[0:72, js]
                        d1 = tpool.tile([msz, N], BF16, name=f"d1_{mc}",
                                        tag=f"d1_{mc}")
                        nc.vector.tensor_sub(d1[:], hs, nn[:])
                        nc.gpsimd.tensor_mul(d1[:], z16g[mc][:], d1[:])
                        nc.gpsimd.tensor_add(hs, nn[:], d1[:])
                        if step < N_STEPS - 1:
                            if mc == 0:
                                nc.vector.tensor_copy(h8[:, 0, js], hs)
                            else:
                                nc.vector.tensor_copy(h8[0:72, 1, js], hs)

                # Cross-step software pipeline at graph granularity: while
                # step s-1's per-graph gate/update chains drain on
                # ACT/DVE/GpSimd, the PE runs step s's Ht/aggregation for
                # graphs two behind.  s ranges over 0..N_STEPS; iteration s
                # issues phaseB(s-1, *) interleaved with phaseA(s, *).
                htgs = [None] * G
                for s in range(N_STEPS + 1):
                    if s == 0:
                        htgs[0] = phaseA_ht(0)
                        for g in range(G):
                            if g + 1 < G:
                                htgs[g + 1] = phaseA_ht(g + 1)
                            phaseA_agg(g, htgs[g])
                    elif s < N_STEPS:
                        for g in range(G):
                            phaseB(g, s - 1)
                            if g >= 2:
                                htgs[g - 2] = phaseA_ht(g - 2)
                            if g >= 3:
                                phaseA_agg(g - 3, htgs[g - 3])
                        htgs[G - 2] = phaseA_ht(G - 2)
                        phaseA_agg(G - 3, htgs[G - 3])
                        htgs[G - 1] = phaseA_ht(G - 1)
                        phaseA_agg(G - 2, htgs[G - 2])
                        phaseA_agg(G - 1, htgs[G - 1])
                    else:
                        for g in range(G):
                            phaseB(g, N_STEPS - 1)

            # ============ head ============
            with (
                tc.tile_pool(name="head", bufs=1) as hpool,
                tc.tile_pool(name="htmp", bufs=2) as htp,
            ):
                L1 = N - 2          # 510
                P1 = 254
                P2 = 127
                y1p = [hpool.tile([sz, G * P1], BF16, name=f"y1p{i}")
                       for i, (_, sz) in enumerate(OC)]
                z1p = [hpool.tile([sz, G * P1], BF16, name=f"z1p{i}")
                       for i, (_, sz) in enumerate(CC)]
                y2p = [hpool.tile([sz, G * P2], BF16, name=f"y2p{i}")
                       for i, (_, sz) in enumerate(AC)]
                z2p = [hpool.tile([sz, G * P2], BF16, name=f"z2p{i}")
                       for i, (_, sz) in enumerate(MZC)]
                avg7 = hpool.tile([N_CLASSES, G], F32, name="avg7")

                # cT1 rows 72:128 (= x rows 0:56) and cT2 were staged at init;
                # fill cT1 rows 0:72 (= final h rows 128:200) per graph on
                # GpSimd so the DVE FIFO isn't head-of-line blocked.
                nc.vector.memset(y2p[1][:], 1.0)
                nc.vector.memset(z2p[2][:], 1.0)
                for g in range(G):
                    js = slice(g * N, (g + 1) * N)
                    nc.vector.tensor_copy(cT1[0:72, js], hm1[0:72, js])
                yin = [hm0, hm1]
                yinsz = [128, 72]
                zin = [hm0, cT1, cT2]
                zinsz = [128, 128, 44]

                def branch(nco, cin, cinsz, w1, b1, w2, b2, p1t, p2t, och, pfx):
                    # stationary-major: one weight load per (k, ci) feeds all
                    # G graphs (8 live PSUM banks), so LDWEIGHTS fully hides.
                    for co, (cooff, cosz) in enumerate(och):
                        pcs = [ps(cosz, L1) for _ in range(G)]
                        kk = 0
                        nk = 3 * len(cin)
                        for k in range(3):
                            for ci in range(len(cin)):
                                for g in range(G):
                                    nc.tensor.matmul(
                                        pcs[g][:],
                                        w1[ci][:, k * nco + cooff:
                                               k * nco + cooff + cosz],
                                        cin[ci][0:cinsz[ci],
                                                g * N + k:g * N + k + L1],
                                        start=(kk == 0), stop=(kk == nk - 1))
                                kk += 1
                        for g in range(G):
                            rl = htp.tile([cosz, L1], BF16, name=f"rl{pfx}{co}",
                                          tag=f"rl{pfx}{co}")
                            nc.scalar.activation(
                                rl[:], pcs[g][:],
                                mybir.ActivationFunctionType.Relu,
                                bias=b1[co][:])
                            m1 = htp.tile([cosz, P1], BF16, name=f"m1{pfx}{co}",
                                          tag=f"m1{pfx}{co}")
                            nc.vector.tensor_max(m1[:], rl[:, 0:508:2],
                                                 rl[:, 1:509:2])
                            nc.vector.tensor_max(
                                p1t[co][0:cosz, g * P1:(g + 1) * P1],
                                m1[:], rl[:, 2:510:2])
                    for co, (cooff, cosz) in enumerate(och):
                        pcs = [ps(cosz, P1) for _ in range(G)]
                        for ci, (cioff, cisz) in enumerate(och):
                            for g in range(G):
                                nc.tensor.matmul(
                                    pcs[g][:],
                                    w2[ci][:, cooff:cooff + cosz],
                                    p1t[ci][0:cisz, g * P1:(g + 1) * P1],
                                    start=(ci == 0), stop=(ci == len(och) - 1))
                        for g in range(G):
                            rl = htp.tile([cosz, P1], BF16, name=f"rw{pfx}{co}",
                                          tag=f"rw{pfx}{co}")
                            nc.scalar.activation(
                                rl[:], pcs[g][:],
                                mybir.ActivationFunctionType.Relu,
                                bias=b2[co][:])
                            nc.vector.tensor_max(
                                p2t[co][0:cosz, g * P2:(g + 1) * P2],
                                rl[:, 0:254:2], rl[:, 1:254:2])

                branch(OUT, yin, yinsz, c1w, c1b, c2w, c2b, y1p, y2p, OC, "y")
                branch(CONCAT, zin, zinsz, cc1w, cc1b, cc2w, cc2b, z1p, z2p,
                       CC, "z")

                # mlps + product + mean over positions
                for g in range(G):
                    gs = slice(g * P2, (g + 1) * P2)
                    pby = ps(N_CLASSES, P2)
                    for k, (off, sz) in enumerate(AC):
                        nc.tensor.matmul(pby[:], mlpy[k][:], y2p[k][0:sz, gs],
                                         start=(k == 0), stop=(k == 1))
                    pbz = ps(N_CLASSES, P2)
                    for k, (off, sz) in enumerate(MZC):
                        nc.tensor.matmul(pbz[:], mlpz[k][:], z2p[k][0:sz, gs],
                                         start=(k == 0), stop=(k == 2))
                    t1 = htp.tile([N_CLASSES, P2], F32, name="tm1", tag="tm1")
                    nc.vector.tensor_copy(t1[:], pby[:])
                    nc.vector.tensor_mul(t1[:], t1[:], pbz[:])
                    nc.vector.reduce_sum(avg7[:, g:g + 1], t1[:],
                                         axis=mybir.AxisListType.X)

                # transpose [7, G] -> [G, 7], softmax rows, write out
                pT = ps(G, N_CLASSES)
                nc.tensor.transpose(pT[:], avg7[:],
                                    ident[0:N_CLASSES, 0:N_CLASSES])
                sm = htp.tile([G, N_CLASSES], F32, name="sm", tag="sm")
                nc.vector.tensor_copy(sm[:], pT[:])
                mx = htp.tile([G, 1], F32, name="mx", tag="mx")
                nc.vector.reduce_max(mx[:], sm[:], axis=mybir.AxisListType.X)
                nmx = htp.tile([G, 1], F32, name="nmx", tag="nmx")
                nc.vector.tensor_scalar_mul(nmx[:], mx[:], -1.0)
                ex = htp.tile([G, N_CLASSES], F32, name="ex", tag="ex")
                ssum = htp.tile([G, 1], F32, name="ssum", tag="ssum")
                nc.scalar.activation(ex[:], sm[:],
                                     mybir.ActivationFunctionType.Exp,
                                     bias=nmx[:], accum_out=ssum[:])
                rec = htp.tile([G, 1], F32, name="rec", tag="rec")
                nc.vector.reciprocal(rec[:], ssum[:])
                fin = htp.tile([G, N_CLASSES], F32, name="fin", tag="fin")
                nc.vector.tensor_scalar_mul(fin[:], ex[:], rec[:])
                nc.sync.dma_start(d_out[:], fin[:])

    nc.compile()
    return nc


_NC = None


def _get_nc():
    global _NC
    if _NC is None:
        _NC = _build_program()
    return _NC


def _prep_inputs(features, edge_src, edge_dst, edge_types, etype_W, etype_b,
                 W_ih, b_ih, W_hh, b_hh, conv1_w, conv1_b, conv2_w, conv2_b,
                 convc1_w, convc1_b, convc2_w, convc2_b,
                 mlp_y_w, mlp_y_b, mlp_z_w, mlp_z_b):
    bf = ml_dtypes.bfloat16
    f8 = ml_dtypes.float8_e4m3
    f32 = np.float32

    # dense adjacency counts: adj[graph, src%128, t*4 + src//128, dst]
    gid = (edge_src // N).astype(np.int64)
    s = (edge_src % N).astype(np.int64)
    d = (edge_dst % N).astype(np.int64)
    t = edge_types.astype(np.int64)
    adj = np.zeros((B, 128, 16, N), np.float32)
    np.add.at(adj, (gid, s % 128, t * 4 + s // 128, d), 1.0)
    adj = adj.astype(f8)

    wcat = np.zeros((OUT + 1, NT * OUT), f32)
    for tt in range(NT):
        wcat[:OUT, tt * OUT:(tt + 1) * OUT] = etype_W[tt].T
        wcat[OUT, tt * OUT:(tt + 1) * OUT] = etype_b[tt]
    wih = np.concatenate([W_ih.T, b_ih[None, :]], 0)
    whhT = np.concatenate([W_hh.T, b_hh[None, :]], 0)   # [201, 600]
    whh8 = np.zeros((128, 2, WPAD), f32)
    whh8[:, 0, 0:3 * OUT] = whhT[0:128]
    whh8[0:73, 1, 0:3 * OUT] = whhT[128:201]
    c1w = conv1_w.transpose(1, 2, 0).reshape(OUT, 3 * OUT)
    c2w = conv2_w[:, :, 0].T
    cc1w = convc1_w.transpose(1, 2, 0).reshape(CONCAT, 3 * CONCAT)
    cc2w = convc2_w[:, :, 0].T
    s127 = np.float32(1.0 / np.sqrt(127.0))
    mlpy = np.concatenate([mlp_y_w.T, mlp_y_b[None, :]], 0) * s127
    mlpz = np.concatenate([mlp_z_w.T, mlp_z_b[None, :]], 0) * s127
    ident = np.eye(128, dtype=f32)

    shared = dict(
        wcat=wcat.astype(bf), wih=wih.astype(bf), whh8=whh8.astype(f8),
        c1w=c1w.astype(bf), c2w=c2w.astype(bf),
        cc1w=cc1w.astype(bf), cc2w=cc2w.astype(bf),
        mlpy=mlpy.astype(bf), mlpz=mlpz.astype(bf),
        c1b=conv1_b.reshape(-1, 1).astype(f32),
        c2b=conv2_b.reshape(-1, 1).astype(f32),
        cc1b=convc1_b.reshape(-1, 1).astype(f32),
        cc2b=convc2_b.reshape(-1, 1).astype(f32),
        ident=ident,
        ones8=np.ones((1, NLOC), np.float32).astype(f8),
    )
    in_maps = []
    for c in range(N_CORES):
        m = dict(shared)
        m["features"] = np.ascontiguousarray(features[c * NLOC:(c + 1) * NLOC]).astype(f32)
        m["adj"] = np.ascontiguousarray(adj[c * G:(c + 1) * G])
        in_maps.append(m)
    return in_maps


def run(inputs, trace=False):
    nc = _get_nc()
    in_maps = _prep_inputs(**inputs)
    res = run_bass_kernel_spmd(nc, in_maps, core_ids=list(range(N_CORES)),
                               trace=trace)
    out = np.concatenate([res.results[i]["out"] for i in range(N_CORES)], 0)
    return out.astype(np.float32), res


def kernel(**inputs):
    out, _ = run(inputs, trace=False)
    return out
